# revision 1
# baseline (speedup 1.0000x reference)
"""Component Heston-Nandi GARCH volatility recurrence on 8 Trainium2 cores.

Strategy: the (h,q) recurrence is strongly contracting (empirical rate ~0.981
per step), so the 1M-step sequential scan is split into 8192 chunks of C=128
steps, each computed by one SIMD lane (8 cores x 128 partitions x F=8 free
lanes).  Each lane warms up for W steps before its chunk starts, from a
stationary initial guess, which converges its state to fp32 accuracy.  Lanes
whose chunk starts before position W instead start *exactly* at t=0 via
synthetic fixed-point warmup data, so early outputs are exact.

The q-state is eliminated algebraically: with
    bA=(1-phi)vphi+alpha, bu=-2[(1-phi)vphi gam2 + alpha gam1]
    c1=phi+rho+bA lam^2-bu lam, c2=-rho(phi+alpha lam^2+2 alpha gam1 lam)
    c4=-rho alpha, nu=-c4/bA, k1=c1-nu, gam=c2+nu k1
the recurrence becomes, per lane (all fp32 on device):
    h_{t+1} = bA*y_t^2 * (1/h_t) + P_t ;  P_t = k1*h_t + Q_{t-1}
    Q_t     = gam*h_t + nu*Q_{t-1} + D_{t+1}
    D_t     = e1*y_t + e2*y_{t-1} + K2          (precomputed on host)
Per step: 4 DVE ops (reciprocal, mult, stt, add) + 2 Pool ops (Q update).
"""
import numpy as np

T = 1048576
NCORES = 8
F = 16           # lanes per partition (free dim)
C = T // (NCORES * 128 * F)   # chunk length per lane
W = 512          # warmup steps
NSTEP = W + C - 1
SEG = 64         # steps per hbuf segment tile
DMASLICE = 128   # steps per input DMA slice; slices stream in behind the
                 # compute (Bacc's generate_event_semaphores legalizes the
                 # multi-wait instructions this creates)

_cache = {}


def _build():
    import concourse.bacc as bacc
    import concourse.mybir as mybir
    from concourse.tile import TileContext

    f32 = mybir.dt.float32
    add = mybir.AluOpType.add
    mult = mybir.AluOpType.mult

    # Bacc (not raw Bass): its finalize() runs generate_event_semaphores,
    # which splits multi-wait instructions to the HW's 1-wait-per-inst limit.
    nc = bacc.Bacc("TRN2", target_bir_lowering=False, debug=False,
                   num_devices=NCORES)
    # single input blob keeps the total DMA count (and thus distinct HWDGE
    # queue sems the kernel-tail drain waits on) small:
    # [ aux(2F+3) | y2_s0 | dd_s0 | y2_s1 | dd_s1 | ... ] slice-interleaved
    AUXW = 2 * F + 3
    blob_in = nc.dram_tensor("blob", [128, AUXW + 2 * NSTEP * F], f32,
                             kind="ExternalInput")
    out = nc.dram_tensor("o", [128, F * C], f32, kind="ExternalOutput")

    nseg = (NSTEP + 1 + SEG - 1) // SEG
    ndma = (NSTEP + DMASLICE - 1) // DMASLICE

    with TileContext(nc) as tc:
        with (
            tc.tile_pool(name="data", bufs=1) as dpool,
            tc.tile_pool(name="state", bufs=1) as spool,
        ):
            def slice_steps(i):
                return min(DMASLICE, NSTEP - i * DMASLICE)
            blob = [dpool.tile(
                [128, (AUXW if i == 0 else 0) + 2 * slice_steps(i) * F], f32,
                name=f"blob_{i}", tag=f"blob_{i}") for i in range(ndma)]
            aux = blob[0][:, 0:AUXW]
            hseg = [spool.tile([128, SEG * F], f32, name=f"h_{i}", tag=f"h_{i}")
                    for i in range(nseg)]
            hout = spool.tile([128, F * C], f32, name="hout", tag="hout")
            NQ = 8
            qb = [spool.tile([128, F], f32, name=f"q_{i}", tag=f"q_{i}") for i in range(NQ)]
            NR = 4
            rb = [spool.tile([128, F], f32, name=f"r_{i}", tag=f"r_{i}") for i in range(NR)]
            mb = [spool.tile([128, F], f32, name=f"m_{i}", tag=f"m_{i}") for i in range(NR)]
            pb = [spool.tile([128, F], f32, name=f"p_{i}", tag=f"p_{i}") for i in range(NR)]
            qa = [spool.tile([128, F], f32, name=f"qa_{i}", tag=f"qa_{i}") for i in range(NR)]
            ytch = [spool.tile([128, 1], f32, name=f"yt_{i}", tag=f"yt_{i}")
                    for i in range(ndma)]
            dtch = [spool.tile([128, 1], f32, name=f"dt_{i}", tag=f"dt_{i}")
                    for i in range(ndma)]

            off = 0
            for i in range(ndma):
                w = (AUXW if i == 0 else 0) + 2 * slice_steps(i) * F
                nc.sync.dma_start(blob[i][:], blob_in[:, off:off + w])
                off += w

            k1_ap = aux[:, 2 * F:2 * F + 1]
            nu_ap = aux[:, 2 * F + 1:2 * F + 2]
            gam_ap = aux[:, 2 * F + 2:2 * F + 3]

            def hcol(j):  # h at step j lives in segment j//SEG, col j%SEG
                s, o = divmod(j, SEG)
                return hseg[s][:, o * F:(o + 1) * F]

            def y2col(j):
                s, o = divmod(j, DMASLICE)
                base = AUXW if s == 0 else 0
                return blob[s][:, base + o * F:base + (o + 1) * F]

            def ddcol(j):
                s, o = divmod(j, DMASLICE)
                base = (AUXW if s == 0 else 0) + slice_steps(s) * F
                return blob[s][:, base + o * F:base + (o + 1) * F]

            # init: h_0 and Q_{-1}; touch first input slices (absorbs their DMA
            # waits into single-wait TensorCopy instructions — the STT/TT ISA
            # structs only have room for ONE sync-wait command each).
            nc.vector.tensor_copy(hcol(0), aux[:, 0:F])
            nc.vector.tensor_copy(qb[(NQ - 1) % NQ][:], aux[:, F:2 * F])
            nc.vector.tensor_copy(ytch[0][:], blob[0][:, 0:1])
            nc.vector.tensor_copy(dtch[0][:], blob[0][:, 1:2])

            for j in range(NSTEP):
                Hj = hcol(j)
                Hn = hcol(j + 1)
                Qp = qb[(j - 1) % NQ][:]
                Qn = qb[j % NQ][:]
                r = rb[j % NR][:]
                m = mb[j % NR][:]
                P = pb[j % NR][:]
                Qa = qa[j % NR][:]
                nc.vector.reciprocal(r, Hj)
                nc.vector.scalar_tensor_tensor(P, Hj, k1_ap, Qp, mult, add)
                nc.vector.scalar_tensor_tensor(m, r, 1.0, y2col(j), mult, mult)
                nc.vector.scalar_tensor_tensor(Hn, m, 1.0, P, mult, add)
                if j < NSTEP - 1:
                    # Qa carries the cross-engine (DVE h) wait; Qn is then
                    # Pool-local so each op needs exactly one wait.
                    nc.vector.scalar_tensor_tensor(Qa, Hj, gam_ap, ddcol(j),
                                                   mult, add)
                    nc.vector.scalar_tensor_tensor(Qn, Qp, nu_ap, Qa, mult, add)
                # touch the next input slices a few steps before first use
                if j % DMASLICE == DMASLICE - 8:
                    i = j // DMASLICE + 1
                    if i < ndma:
                        nc.vector.tensor_copy(ytch[i][:], blob[i][:, 0:1])
                        nc.vector.tensor_copy(dtch[i][:], blob[i][:, 1:2])

            # layout fix: hbuf (j-major) -> hout (lane-major, contiguous per lane)
            # output h for lane f at out-col f*C + jj', source step j = W + jj' - ... :
            # out index jj' in [0,C) corresponds to h column (W-1+jj')+1 = W+jj'
            for f in range(F):
                # gather C values: hcol(W+jj')[:, f] for jj' in 0..C-1
                # source AP: stride F within each segment; segments are separate
                # tiles, so do it per segment piece.
                jj = 0
                while jj < C:
                    j = W + jj
                    s, o = divmod(j, SEG)
                    n = min(C - jj, SEG - o)
                    src = hseg[s][:].rearrange("p (t f) -> p t f", f=F)[:, o:o + n, f]
                    nc.vector.tensor_copy(hout[:, f * C + jj:f * C + jj + n], src)
                    jj += n

            nc.sync.dma_start(out[:], hout[:])
    nc.finalize()
    return nc


def _prep_inputs(y, omega, alpha, phi, lam, gam1, gam2, vphi, rho):
    """Host-side per-core input construction (fp64 intermediate)."""
    y = np.asarray(y, dtype=np.float32)
    bA = (1 - phi) * vphi + alpha
    bu = -2 * ((1 - phi) * vphi * gam2 + alpha * gam1)
    c1 = phi + rho + bA * lam**2 - bu * lam
    c2 = -rho * (phi + alpha * lam**2 + 2 * alpha * gam1 * lam)
    c4 = -rho * alpha
    K2 = (1 - phi) * (1 - rho) * omega - (1 - phi) * vphi - alpha * (1 - rho)
    e1 = bu - 2 * bA * lam
    e2 = 2 * rho * alpha * (lam + gam1)
    nu = -c4 / bA
    k1 = c1 - nu
    gam = c2 + nu * k1
    Kc = (1 - phi) * omega * (1 - rho) - (1 - phi) * vphi - alpha
    cP = phi + bA * lam**2 - bu * lam

    q0 = float(np.var(y.astype(np.float64)))
    yq = y.astype(np.float64)
    y2 = yq * yq

    # global lane table: lane g = (core*128 + p)*F + f ; chunkstart = g*C
    G = NCORES * 128 * F
    s = np.arange(G) * C
    j = np.arange(NSTEP)
    iy = s[:, None] - W + j[None, :]          # [G, NSTEP]
    iy_c = np.clip(iy, 0, T - 1)
    iy1_c = np.clip(iy + 1, 0, T - 1)
    Y2 = (bA * y2[iy_c]).astype(np.float32)
    DD = (e1 * yq[iy1_c] + e2 * yq[iy_c] + K2).astype(np.float32)

    Pstar = q0 * (1 - bA)
    Qstar = Pstar - k1 * q0
    Dstar = Qstar * (1 - nu) - gam * q0
    syn = iy < -1
    Y2[syn] = np.float32(bA * q0 * q0)
    DD[syn] = np.float32(Dstar)
    tr = iy == -1
    Y2[tr] = np.float32(bA * q0 * q0)
    P0_exact = cP * q0 + (1 - phi) * rho * q0 + e1 * yq[0] + Kc
    D0_craft = (P0_exact - k1 * q0) - gam * q0 - nu * Qstar
    DD[tr] = np.float32(D0_craft)

    iy0 = s - W
    Pinit = np.where(iy0 >= 0,
                     cP * q0 + (1 - phi) * rho * q0 + e1 * yq[np.clip(iy0, 0, T - 1)] + Kc,
                     Pstar)
    Qinit = (Pinit - k1 * q0).astype(np.float32)
    hinit = np.full(G, q0, dtype=np.float32)

    # reshape to per-core, per-partition, j-major-free layout
    # lane g = (core*128+p)*F + f  ->  Y2core[core][p, jj*F + f]
    Y2 = Y2.reshape(NCORES, 128, F, NSTEP).transpose(0, 1, 3, 2).reshape(
        NCORES, 128, NSTEP * F)
    DD = DD.reshape(NCORES, 128, F, NSTEP).transpose(0, 1, 3, 2).reshape(
        NCORES, 128, NSTEP * F)
    hinit = hinit.reshape(NCORES, 128, F)
    Qinit = Qinit.reshape(NCORES, 128, F)

    in_maps = []
    for k in range(NCORES):
        aux = np.empty((128, 2 * F + 3), dtype=np.float32)
        aux[:, 0:F] = hinit[k]
        aux[:, F:2 * F] = Qinit[k]
        aux[:, 2 * F] = np.float32(k1)
        aux[:, 2 * F + 1] = np.float32(nu)
        aux[:, 2 * F + 2] = np.float32(gam)
        AUXW = 2 * F + 3
        blobk = np.empty((128, AUXW + 2 * NSTEP * F), dtype=np.float32)
        blobk[:, :AUXW] = aux
        off = AUXW
        jlo = 0
        while jlo < NSTEP:
            n = min(DMASLICE, NSTEP - jlo)
            blobk[:, off:off + n * F] = Y2[k][:, jlo * F:(jlo + n) * F]
            off += n * F
            blobk[:, off:off + n * F] = DD[k][:, jlo * F:(jlo + n) * F]
            off += n * F
            jlo += n
        in_maps.append({"blob": blobk})
    return in_maps, np.float32(q0)


def kernel(y, omega, alpha, phi, lam, gam1, gam2, vphi, rho, _timing=None):
    from concourse.bass_utils import run_bass_kernel_spmd

    in_maps, q0 = _prep_inputs(
        y, float(omega), float(alpha), float(phi), float(lam),
        float(gam1), float(gam2), float(vphi), float(rho))

    if "nc" not in _cache:
        _cache["nc"] = _build()
    nc = _cache["nc"]

    trace = _timing is not None
    res = run_bass_kernel_spmd(nc, in_maps, core_ids=list(range(NCORES)),
                               trace=trace)
    if trace:
        _timing["exec_time_ns"] = res.exec_time_ns

    outp = np.empty(T, dtype=np.float32)
    for k in range(NCORES):
        outp[k * (T // NCORES):(k + 1) * (T // NCORES)] = \
            res.results[k]["o"].reshape(-1)
    outp[0] = q0
    return outp



# revision 3
# speedup vs baseline: 1.4740x; 1.4740x over previous
"""Component Heston-Nandi GARCH volatility recurrence on 8 Trainium2 cores.

Strategy: the (h,q) recurrence is strongly contracting (empirical rate ~0.981
per step), so the 1M-step sequential scan is split into 8192 chunks of C=128
steps, each computed by one SIMD lane (8 cores x 128 partitions x F=8 free
lanes).  Each lane warms up for W steps before its chunk starts, from a
stationary initial guess, which converges its state to fp32 accuracy.  Lanes
whose chunk starts before position W instead start *exactly* at t=0 via
synthetic fixed-point warmup data, so early outputs are exact.

The q-state is eliminated algebraically: with
    bA=(1-phi)vphi+alpha, bu=-2[(1-phi)vphi gam2 + alpha gam1]
    c1=phi+rho+bA lam^2-bu lam, c2=-rho(phi+alpha lam^2+2 alpha gam1 lam)
    c4=-rho alpha, nu=-c4/bA, k1=c1-nu, gam=c2+nu k1
the recurrence becomes, per lane (all fp32 on device):
    h_{t+1} = bA*y_t^2 * (1/h_t) + P_t ;  P_t = k1*h_t + Q_{t-1}
    Q_t     = gam*h_t + nu*Q_{t-1} + D_{t+1}
    D_t     = e1*y_t + e2*y_{t-1} + K2          (precomputed on host)
Per step: 4 DVE ops (reciprocal, mult, stt, add) + 2 Pool ops (Q update).
"""
import numpy as np

T = 1048576
NCORES = 8
F = 16           # lanes per partition (free dim)
C = T // (NCORES * 128 * F)   # chunk length per lane
W = 320          # warmup steps (host-validated: max rel err 7.6e-3 < 2e-2 gate)
NSTEP = W + C - 1
SEG = 64         # steps per hbuf segment tile
DMASLICE = 128   # steps per input DMA slice; slices stream in behind the
                 # compute (Bacc's generate_event_semaphores legalizes the
                 # multi-wait instructions this creates)

_cache = {}


def _build():
    import concourse.bacc as bacc
    import concourse.mybir as mybir
    from concourse.tile import TileContext

    f32 = mybir.dt.float32
    add = mybir.AluOpType.add
    mult = mybir.AluOpType.mult

    # Bacc (not raw Bass): its finalize() runs generate_event_semaphores,
    # which splits multi-wait instructions to the HW's 1-wait-per-inst limit.
    nc = bacc.Bacc("TRN2", target_bir_lowering=False, debug=False,
                   num_devices=NCORES)
    # single input blob keeps the total DMA count (and thus distinct HWDGE
    # queue sems the kernel-tail drain waits on) small:
    # [ aux(2F+3) | y2_s0 | dd_s0 | y2_s1 | dd_s1 | ... ] slice-interleaved
    AUXW = 2 * F + 3
    blob_in = nc.dram_tensor("blob", [128, AUXW + 2 * NSTEP * F], f32,
                             kind="ExternalInput")
    out = nc.dram_tensor("o", [128, F * C], f32, kind="ExternalOutput")

    nseg = (NSTEP + 1 + SEG - 1) // SEG
    ndma = (NSTEP + DMASLICE - 1) // DMASLICE

    with TileContext(nc) as tc:
        with (
            tc.tile_pool(name="data", bufs=1) as dpool,
            tc.tile_pool(name="state", bufs=1) as spool,
        ):
            def slice_steps(i):
                return min(DMASLICE, NSTEP - i * DMASLICE)
            blob = [dpool.tile(
                [128, (AUXW if i == 0 else 0) + 2 * slice_steps(i) * F], f32,
                name=f"blob_{i}", tag=f"blob_{i}") for i in range(ndma)]
            aux = blob[0][:, 0:AUXW]
            hseg = [spool.tile([128, SEG * F], f32, name=f"h_{i}", tag=f"h_{i}")
                    for i in range(nseg)]
            hout = spool.tile([128, F * C], f32, name="hout", tag="hout")
            NQ = 8
            qb = [spool.tile([128, F], f32, name=f"q_{i}", tag=f"q_{i}") for i in range(NQ)]
            NR = 4
            rb = [spool.tile([128, F], f32, name=f"r_{i}", tag=f"r_{i}") for i in range(NR)]
            mb = [spool.tile([128, F], f32, name=f"m_{i}", tag=f"m_{i}") for i in range(NR)]
            pb = [spool.tile([128, F], f32, name=f"p_{i}", tag=f"p_{i}") for i in range(NR)]
            qa = [spool.tile([128, F], f32, name=f"qa_{i}", tag=f"qa_{i}") for i in range(NR)]
            ytch = [spool.tile([128, 1], f32, name=f"yt_{i}", tag=f"yt_{i}")
                    for i in range(ndma)]
            dtch = [spool.tile([128, 1], f32, name=f"dt_{i}", tag=f"dt_{i}")
                    for i in range(ndma)]

            off = 0
            for i in range(ndma):
                w = (AUXW if i == 0 else 0) + 2 * slice_steps(i) * F
                nc.sync.dma_start(blob[i][:], blob_in[:, off:off + w])
                off += w

            k1_ap = aux[:, 2 * F:2 * F + 1]
            nu_ap = aux[:, 2 * F + 1:2 * F + 2]
            gam_ap = aux[:, 2 * F + 2:2 * F + 3]

            def hcol(j):  # h at step j lives in segment j//SEG, col j%SEG
                s, o = divmod(j, SEG)
                return hseg[s][:, o * F:(o + 1) * F]

            def y2col(j):
                s, o = divmod(j, DMASLICE)
                base = AUXW if s == 0 else 0
                return blob[s][:, base + o * F:base + (o + 1) * F]

            def ddcol(j):
                s, o = divmod(j, DMASLICE)
                base = (AUXW if s == 0 else 0) + slice_steps(s) * F
                return blob[s][:, base + o * F:base + (o + 1) * F]

            # init: h_0 and Q_{-1}; touch first input slices (absorbs their DMA
            # waits into single-wait TensorCopy instructions — the STT/TT ISA
            # structs only have room for ONE sync-wait command each).
            nc.vector.tensor_copy(hcol(0), aux[:, 0:F])
            nc.vector.tensor_copy(qb[(NQ - 1) % NQ][:], aux[:, F:2 * F])
            nc.vector.tensor_copy(ytch[0][:], blob[0][:, 0:1])
            nc.vector.tensor_copy(dtch[0][:], blob[0][:, 1:2])

            for j in range(NSTEP):
                Hj = hcol(j)
                Hn = hcol(j + 1)
                Qp = qb[(j - 1) % NQ][:]
                Qn = qb[j % NQ][:]
                r = rb[j % NR][:]
                m = mb[j % NR][:]
                P = pb[j % NR][:]
                Qa = qa[j % NR][:]
                # critical cycle: Hn_{j-1} -> r -> m -> Hn_j.  Issue r first,
                # then the ops that only need Hj (P, Qa) so their issue time
                # hides r's pipeline latency; m then Qn (hides m->Hn), Hn last.
                nc.vector.reciprocal_approx_fast(r, Hj)
                nc.vector.scalar_tensor_tensor(P, Hj, k1_ap, Qp, mult, add)
                if j < NSTEP - 1:
                    nc.vector.scalar_tensor_tensor(Qa, Hj, gam_ap, ddcol(j),
                                                   mult, add)
                nc.vector.scalar_tensor_tensor(m, r, 1.0, y2col(j), mult, mult)
                if j < NSTEP - 1:
                    nc.vector.scalar_tensor_tensor(Qn, Qp, nu_ap, Qa, mult, add)
                nc.vector.scalar_tensor_tensor(Hn, m, 1.0, P, mult, add)
                # touch the next input slices a few steps before first use
                if j % DMASLICE == DMASLICE - 8:
                    i = j // DMASLICE + 1
                    if i < ndma:
                        nc.vector.tensor_copy(ytch[i][:], blob[i][:, 0:1])
                        nc.vector.tensor_copy(dtch[i][:], blob[i][:, 1:2])

            # layout fix: hbuf (j-major) -> hout (lane-major, contiguous per lane)
            # output h for lane f at out-col f*C + jj', source step j = W + jj' - ... :
            # out index jj' in [0,C) corresponds to h column (W-1+jj')+1 = W+jj'
            for f in range(F):
                # gather C values: hcol(W+jj')[:, f] for jj' in 0..C-1
                # source AP: stride F within each segment; segments are separate
                # tiles, so do it per segment piece.
                jj = 0
                while jj < C:
                    j = W + jj
                    s, o = divmod(j, SEG)
                    n = min(C - jj, SEG - o)
                    src = hseg[s][:].rearrange("p (t f) -> p t f", f=F)[:, o:o + n, f]
                    nc.vector.tensor_copy(hout[:, f * C + jj:f * C + jj + n], src)
                    jj += n

            nc.sync.dma_start(out[:], hout[:])
    nc.finalize()
    return nc


def _prep_inputs(y, omega, alpha, phi, lam, gam1, gam2, vphi, rho):
    """Host-side per-core input construction (fp64 intermediate)."""
    y = np.asarray(y, dtype=np.float32)
    bA = (1 - phi) * vphi + alpha
    bu = -2 * ((1 - phi) * vphi * gam2 + alpha * gam1)
    c1 = phi + rho + bA * lam**2 - bu * lam
    c2 = -rho * (phi + alpha * lam**2 + 2 * alpha * gam1 * lam)
    c4 = -rho * alpha
    K2 = (1 - phi) * (1 - rho) * omega - (1 - phi) * vphi - alpha * (1 - rho)
    e1 = bu - 2 * bA * lam
    e2 = 2 * rho * alpha * (lam + gam1)
    nu = -c4 / bA
    k1 = c1 - nu
    gam = c2 + nu * k1
    Kc = (1 - phi) * omega * (1 - rho) - (1 - phi) * vphi - alpha
    cP = phi + bA * lam**2 - bu * lam

    q0 = float(np.var(y.astype(np.float64)))
    yq = y.astype(np.float64)
    y2 = yq * yq

    # global lane table: lane g = (core*128 + p)*F + f ; chunkstart = g*C
    G = NCORES * 128 * F
    s = np.arange(G) * C
    j = np.arange(NSTEP)
    iy = s[:, None] - W + j[None, :]          # [G, NSTEP]
    iy_c = np.clip(iy, 0, T - 1)
    iy1_c = np.clip(iy + 1, 0, T - 1)
    Y2 = (bA * y2[iy_c]).astype(np.float32)
    DD = (e1 * yq[iy1_c] + e2 * yq[iy_c] + K2).astype(np.float32)

    Pstar = q0 * (1 - bA)
    Qstar = Pstar - k1 * q0
    Dstar = Qstar * (1 - nu) - gam * q0
    syn = iy < -1
    Y2[syn] = np.float32(bA * q0 * q0)
    DD[syn] = np.float32(Dstar)
    tr = iy == -1
    Y2[tr] = np.float32(bA * q0 * q0)
    P0_exact = cP * q0 + (1 - phi) * rho * q0 + e1 * yq[0] + Kc
    D0_craft = (P0_exact - k1 * q0) - gam * q0 - nu * Qstar
    DD[tr] = np.float32(D0_craft)

    iy0 = s - W
    Pinit = np.where(iy0 >= 0,
                     cP * q0 + (1 - phi) * rho * q0 + e1 * yq[np.clip(iy0, 0, T - 1)] + Kc,
                     Pstar)
    Qinit = (Pinit - k1 * q0).astype(np.float32)
    hinit = np.full(G, q0, dtype=np.float32)

    # reshape to per-core, per-partition, j-major-free layout
    # lane g = (core*128+p)*F + f  ->  Y2core[core][p, jj*F + f]
    Y2 = Y2.reshape(NCORES, 128, F, NSTEP).transpose(0, 1, 3, 2).reshape(
        NCORES, 128, NSTEP * F)
    DD = DD.reshape(NCORES, 128, F, NSTEP).transpose(0, 1, 3, 2).reshape(
        NCORES, 128, NSTEP * F)
    hinit = hinit.reshape(NCORES, 128, F)
    Qinit = Qinit.reshape(NCORES, 128, F)

    in_maps = []
    for k in range(NCORES):
        aux = np.empty((128, 2 * F + 3), dtype=np.float32)
        aux[:, 0:F] = hinit[k]
        aux[:, F:2 * F] = Qinit[k]
        aux[:, 2 * F] = np.float32(k1)
        aux[:, 2 * F + 1] = np.float32(nu)
        aux[:, 2 * F + 2] = np.float32(gam)
        AUXW = 2 * F + 3
        blobk = np.empty((128, AUXW + 2 * NSTEP * F), dtype=np.float32)
        blobk[:, :AUXW] = aux
        off = AUXW
        jlo = 0
        while jlo < NSTEP:
            n = min(DMASLICE, NSTEP - jlo)
            blobk[:, off:off + n * F] = Y2[k][:, jlo * F:(jlo + n) * F]
            off += n * F
            blobk[:, off:off + n * F] = DD[k][:, jlo * F:(jlo + n) * F]
            off += n * F
            jlo += n
        in_maps.append({"blob": blobk})
    return in_maps, np.float32(q0)


def kernel(y, omega, alpha, phi, lam, gam1, gam2, vphi, rho, _timing=None):
    from concourse.bass_utils import run_bass_kernel_spmd

    in_maps, q0 = _prep_inputs(
        y, float(omega), float(alpha), float(phi), float(lam),
        float(gam1), float(gam2), float(vphi), float(rho))

    if "nc" not in _cache:
        _cache["nc"] = _build()
    nc = _cache["nc"]

    trace = _timing is not None
    res = run_bass_kernel_spmd(nc, in_maps, core_ids=list(range(NCORES)),
                               trace=trace)
    if trace:
        _timing["exec_time_ns"] = res.exec_time_ns

    outp = np.empty(T, dtype=np.float32)
    for k in range(NCORES):
        outp[k * (T // NCORES):(k + 1) * (T // NCORES)] = \
            res.results[k]["o"].reshape(-1)
    outp[0] = q0
    return outp



# revision 4
# speedup vs baseline: 1.8443x; 1.2512x over previous
"""Component Heston-Nandi GARCH volatility recurrence on 8 Trainium2 cores.

Strategy: the (h,q) recurrence is strongly contracting (~0.983/step), so the
1M-step sequential scan is split into 16384 chunks of C=64 steps, each
computed by one SIMD lane (8 cores x 128 partitions x F=16 free lanes).  Each
lane warms up for W=320 steps from a stationary initial guess before its
chunk starts (host-validated max rel err 7.6e-3 vs the 2e-2 gate).  Lanes
whose chunk starts before position W start *exactly* at t=0 via synthetic
fixed-point warmup data.

The q-state is eliminated algebraically (see _prep_inputs) giving per step:
    h_{t+1} = bA*y_t^2 * (1/h_t) + P_t ;  P_t = k1*h_t + Q_{t-1}
    Q_t     = gam*h_t + nu*Q_{t-1} + D_t

Scheduling: hand-authored instruction stream on the Vector engine with NO
per-op semaphores.  The DVE pipeline does not interlock same-engine RAW
hazards, but a probe (proto/probe.py) shows one intervening instruction
(distance >= 2) makes reads bit-exact.  The 6-op ring
    [r, P, m, Qa, Hn, Qn]
has every RAW dependency at distance >= 2, so the only semaphores are the
DMA-slice handshakes.  This runs ~2x faster than the semaphore-synced
schedule (per-op waits were 67-230 ns each).
"""
import numpy as np

T = 1048576
NCORES = 8
F = 16           # lanes per partition (free dim)
C = T // (NCORES * 128 * F)   # chunk length per lane (=64)
W = 320          # warmup steps
NSTEP = W + C - 1
SEG = 64         # steps per h ring segment (C == SEG and W % SEG == 0)
DMASLICES = [32, 128, NSTEP - 160]   # input slices: small first for fast start

_cache = {}


def _build():
    import concourse.bacc as bacc
    import concourse.mybir as mybir
    from contextlib import ExitStack

    f32 = mybir.dt.float32
    add = mybir.AluOpType.add
    mult = mybir.AluOpType.mult

    nc = bacc.Bacc("TRN2", target_bir_lowering=False, debug=False,
                   num_devices=NCORES)
    AUXW = 2 * F + 3
    blob_in = nc.dram_tensor("blob", [128, AUXW + 2 * NSTEP * F], f32,
                             kind="ExternalInput")
    out = nc.dram_tensor("o", [128, F * C], f32, kind="ExternalOutput")

    nseg = (NSTEP + SEG) // SEG   # h columns 0..NSTEP inclusive
    nsl = len(DMASLICES)
    sl_start = [0] * nsl
    for i in range(1, nsl):
        sl_start[i] = sl_start[i - 1] + DMASLICES[i - 1]

    NQ = 8
    NR = 4
    with ExitStack() as ctx:
        sems = [ctx.enter_context(nc.semaphore(f"ds{i}")) for i in range(nsl)]
        csem = ctx.enter_context(nc.semaphore("csem"))
        blob = [ctx.enter_context(nc.sbuf_tensor(
            f"blob{i}", [128, (AUXW if i == 0 else 0) + 2 * n * F], f32))
            for i, n in enumerate(DMASLICES)]
        hseg = [ctx.enter_context(nc.sbuf_tensor(f"h{i}", [128, SEG * F], f32))
                for i in range(nseg)]
        hout = ctx.enter_context(nc.sbuf_tensor("hout", [128, F * C], f32))
        qb = [ctx.enter_context(nc.sbuf_tensor(f"q{i}", [128, F], f32))
              for i in range(NQ)]
        rb = [ctx.enter_context(nc.sbuf_tensor(f"r{i}", [128, F], f32))
              for i in range(NR)]
        mb = [ctx.enter_context(nc.sbuf_tensor(f"m{i}", [128, F], f32))
              for i in range(NR)]
        pb = [ctx.enter_context(nc.sbuf_tensor(f"p{i}", [128, F], f32))
              for i in range(NR)]
        qa = [ctx.enter_context(nc.sbuf_tensor(f"qa{i}", [128, F], f32))
              for i in range(NR)]
        pad = ctx.enter_context(nc.sbuf_tensor("pad", [128, F], f32))

        off = 0
        for i, n in enumerate(DMASLICES):
            w = (AUXW if i == 0 else 0) + 2 * n * F
            nc.sync.dma_start(blob[i][:, :], blob_in[:, off:off + w]) \
                .then_inc(sems[i], 16)
            off += w

        aux = blob[0]
        k1_ap = aux[:, 2 * F:2 * F + 1]
        nu_ap = aux[:, 2 * F + 1:2 * F + 2]
        gam_ap = aux[:, 2 * F + 2:2 * F + 3]

        def hcol(j):
            s, o = divmod(j, SEG)
            return hseg[s][:, o * F:(o + 1) * F]

        def sl_of(j):
            for i in range(nsl - 1, -1, -1):
                if j >= sl_start[i]:
                    return i

        def y2col(j):
            s = sl_of(j)
            o = j - sl_start[s]
            base = AUXW if s == 0 else 0
            return blob[s][:, base + o * F:base + (o + 1) * F]

        def ddcol(j):
            s = sl_of(j)
            o = j - sl_start[s]
            base = (AUXW if s == 0 else 0) + DMASLICES[s] * F
            return blob[s][:, base + o * F:base + (o + 1) * F]

        # init: h_0 and Q_{-1} (pad memset keeps first-step RAW distances >= 2)
        nc.vector.wait_ge(sems[0], 16)
        nc.vector.tensor_copy(hcol(0), aux[:, 0:F])
        nc.vector.tensor_copy(qb[(NQ - 1) % NQ][:, :], aux[:, F:2 * F])
        nc.vector.memset(pad[:, :], 0.0)

        for j in range(NSTEP):
            if j in (sl_start[1], sl_start[2]):
                nc.vector.wait_ge(sems[sl_of(j)], 16)
            Hj = hcol(j)
            Hn = hcol(j + 1)
            Qp = qb[(j - 1) % NQ][:, :]
            Qn = qb[j % NQ][:, :]
            r = rb[j % NR][:, :]
            m = mb[j % NR][:, :]
            P = pb[j % NR][:, :]
            Qa = qa[j % NR][:, :]
            # ring [r, P, m, Qa, Hn, Qn]: every RAW dep >= 2 instructions back
            nc.vector.reciprocal_approx_fast(r, Hj)
            nc.vector.scalar_tensor_tensor(P, Hj, k1_ap, Qp, mult, add)
            nc.vector.scalar_tensor_tensor(m, r, 1.0, y2col(j), mult, mult)
            nc.vector.scalar_tensor_tensor(Qa, Hj, gam_ap, ddcol(j), mult, add)
            nc.vector.scalar_tensor_tensor(Hn, m, 1.0, P, mult, add)
            nc.vector.scalar_tensor_tensor(Qn, Qp, nu_ap, Qa, mult, add)

        # layout fix: h columns W..W+C-1 all live in hseg[W//SEG] (C==SEG,
        # W%SEG==0); gather per lane f into contiguous hout columns.
        src_seg = hseg[W // SEG][:, :].rearrange("p (t f) -> p t f", f=F)
        for f in range(F):
            inst = nc.vector.tensor_copy(hout[:, f * C:(f + 1) * C],
                                         src_seg[:, :, f])
        inst.then_inc(csem, 1)
        nc.sync.wait_ge(csem, 1)
        nc.sync.dma_start(out[:, :], hout[:, :]).then_inc(sems[0], 16)
    nc.finalize()
    return nc


def _prep_inputs(y, omega, alpha, phi, lam, gam1, gam2, vphi, rho):
    """Host-side per-core input construction (fp64 intermediate)."""
    y = np.asarray(y, dtype=np.float32)
    bA = (1 - phi) * vphi + alpha
    bu = -2 * ((1 - phi) * vphi * gam2 + alpha * gam1)
    c1 = phi + rho + bA * lam**2 - bu * lam
    c2 = -rho * (phi + alpha * lam**2 + 2 * alpha * gam1 * lam)
    c4 = -rho * alpha
    K2 = (1 - phi) * (1 - rho) * omega - (1 - phi) * vphi - alpha * (1 - rho)
    e1 = bu - 2 * bA * lam
    e2 = 2 * rho * alpha * (lam + gam1)
    nu = -c4 / bA
    k1 = c1 - nu
    gam = c2 + nu * k1
    Kc = (1 - phi) * omega * (1 - rho) - (1 - phi) * vphi - alpha
    cP = phi + bA * lam**2 - bu * lam

    q0 = float(np.var(y.astype(np.float64)))
    yq = y.astype(np.float64)
    y2 = yq * yq

    # global lane table: lane g = (core*128 + p)*F + f ; chunkstart = g*C
    G = NCORES * 128 * F
    s = np.arange(G) * C
    j = np.arange(NSTEP)
    iy = s[:, None] - W + j[None, :]          # [G, NSTEP]
    iy_c = np.clip(iy, 0, T - 1)
    iy1_c = np.clip(iy + 1, 0, T - 1)
    Y2 = (bA * y2[iy_c]).astype(np.float32)
    DD = (e1 * yq[iy1_c] + e2 * yq[iy_c] + K2).astype(np.float32)

    Pstar = q0 * (1 - bA)
    Qstar = Pstar - k1 * q0
    Dstar = Qstar * (1 - nu) - gam * q0
    syn = iy < -1
    Y2[syn] = np.float32(bA * q0 * q0)
    DD[syn] = np.float32(Dstar)
    tr = iy == -1
    Y2[tr] = np.float32(bA * q0 * q0)
    P0_exact = cP * q0 + (1 - phi) * rho * q0 + e1 * yq[0] + Kc
    D0_craft = (P0_exact - k1 * q0) - gam * q0 - nu * Qstar
    DD[tr] = np.float32(D0_craft)

    iy0 = s - W
    Pinit = np.where(iy0 >= 0,
                     cP * q0 + (1 - phi) * rho * q0 + e1 * yq[np.clip(iy0, 0, T - 1)] + Kc,
                     Pstar)
    Qinit = (Pinit - k1 * q0).astype(np.float32)
    hinit = np.full(G, q0, dtype=np.float32)

    # reshape to per-core, per-partition, j-major-free layout
    Y2 = Y2.reshape(NCORES, 128, F, NSTEP).transpose(0, 1, 3, 2).reshape(
        NCORES, 128, NSTEP * F)
    DD = DD.reshape(NCORES, 128, F, NSTEP).transpose(0, 1, 3, 2).reshape(
        NCORES, 128, NSTEP * F)
    hinit = hinit.reshape(NCORES, 128, F)
    Qinit = Qinit.reshape(NCORES, 128, F)

    in_maps = []
    for k in range(NCORES):
        aux = np.empty((128, 2 * F + 3), dtype=np.float32)
        aux[:, 0:F] = hinit[k]
        aux[:, F:2 * F] = Qinit[k]
        aux[:, 2 * F] = np.float32(k1)
        aux[:, 2 * F + 1] = np.float32(nu)
        aux[:, 2 * F + 2] = np.float32(gam)
        AUXW = 2 * F + 3
        blobk = np.empty((128, AUXW + 2 * NSTEP * F), dtype=np.float32)
        blobk[:, :AUXW] = aux
        off = AUXW
        jlo = 0
        for n in DMASLICES:
            blobk[:, off:off + n * F] = Y2[k][:, jlo * F:(jlo + n) * F]
            off += n * F
            blobk[:, off:off + n * F] = DD[k][:, jlo * F:(jlo + n) * F]
            off += n * F
            jlo += n
        in_maps.append({"blob": blobk})
    return in_maps, np.float32(q0)


def kernel(y, omega, alpha, phi, lam, gam1, gam2, vphi, rho, _timing=None):
    from concourse.bass_utils import run_bass_kernel_spmd

    in_maps, q0 = _prep_inputs(
        y, float(omega), float(alpha), float(phi), float(lam),
        float(gam1), float(gam2), float(vphi), float(rho))

    if "nc" not in _cache:
        _cache["nc"] = _build()
    nc = _cache["nc"]

    trace = _timing is not None
    res = run_bass_kernel_spmd(nc, in_maps, core_ids=list(range(NCORES)),
                               trace=trace)
    if trace:
        _timing["exec_time_ns"] = res.exec_time_ns

    outp = np.empty(T, dtype=np.float32)
    for k in range(NCORES):
        outp[k * (T // NCORES):(k + 1) * (T // NCORES)] = \
            res.results[k]["o"].reshape(-1)
    outp[0] = q0
    return outp


# revision 9
# speedup vs baseline: 2.2331x; 1.2108x over previous
"""Component Heston-Nandi GARCH volatility recurrence on 8 Trainium2 cores.

Strategy: the (h,q) recurrence is strongly contracting (~0.983/step), so the
1M-step sequential scan is split into 16384 chunks of C=64 steps, each
computed by one SIMD lane (8 cores x 128 partitions x F=16 free lanes).  Each
lane warms up for W=320 steps from a stationary initial guess before its
chunk starts (host-validated max rel err 7.6e-3 vs the 2e-2 gate).  Lanes
whose chunk starts before position W start *exactly* at t=0 via synthetic
fixed-point warmup data.

The q-state is eliminated algebraically (see _prep_inputs) giving per step:
    h_{t+1} = bA*y_t^2 * (1/h_t) + P_t ;  P_t = k1*h_t + Q_{t-1}
    Q_t     = gam*h_t + nu*Q_{t-1} + D_t

Scheduling: hand-authored instruction stream on the Vector engine with NO
per-op semaphores.  The DVE pipeline does not interlock same-engine RAW
hazards, but a probe (proto/probe.py) shows one intervening instruction
(distance >= 2) makes reads bit-exact.  The 6-op ring
    [r, P, m, Qa, Hn, Qn]
has every RAW dependency at distance >= 2, so the only semaphores are the
DMA-slice handshakes.  This runs ~2x faster than the semaphore-synced
schedule (per-op waits were 67-230 ns each).
"""
import numpy as np

T = 1048576
NCORES = 8
F = 32           # lanes per partition (free dim)
C = T // (NCORES * 128 * F)   # chunk length per lane (=32)
W = 320          # warmup steps
NSTEP = W + C - 1
SEG = 64         # steps per h ring segment (W % SEG == 0, W % SEG + C <= SEG)
DMASLICES = [32, 128, NSTEP - 160]   # input slices: small first for fast start

_cache = {}


def _build(k1, nu, gam):
    import concourse.bacc as bacc
    import concourse.mybir as mybir
    from contextlib import ExitStack

    f32 = mybir.dt.float32
    add = mybir.AluOpType.add
    mult = mybir.AluOpType.mult

    nc = bacc.Bacc("TRN2", target_bir_lowering=False, debug=False,
                   num_devices=NCORES)
    AUXW = 2 * F + 3
    blob_in = nc.dram_tensor("blob", [128, AUXW + 2 * NSTEP * F], f32,
                             kind="ExternalInput")
    out = nc.dram_tensor("o", [128, F * C], f32, kind="ExternalOutput")

    nseg = (NSTEP + SEG) // SEG   # h columns 0..NSTEP inclusive
    nsl = len(DMASLICES)
    sl_start = [0] * nsl
    for i in range(1, nsl):
        sl_start[i] = sl_start[i - 1] + DMASLICES[i - 1]

    NQ = 8
    NR = 4
    with ExitStack() as ctx:
        sems = [ctx.enter_context(nc.semaphore(f"ds{i}")) for i in range(nsl)]
        csem = ctx.enter_context(nc.semaphore("csem"))
        blob = [ctx.enter_context(nc.sbuf_tensor(
            f"blob{i}", [128, (AUXW if i == 0 else 0) + 2 * n * F], f32))
            for i, n in enumerate(DMASLICES)]
        hseg = [ctx.enter_context(nc.sbuf_tensor(f"h{i}", [128, SEG * F], f32))
                for i in range(nseg)]
        hout = ctx.enter_context(nc.sbuf_tensor("hout", [128, F * C], f32))
        qb = [ctx.enter_context(nc.sbuf_tensor(f"q{i}", [128, F], f32))
              for i in range(NQ)]
        rb = [ctx.enter_context(nc.sbuf_tensor(f"r{i}", [128, F], f32))
              for i in range(NR)]
        mb = [ctx.enter_context(nc.sbuf_tensor(f"m{i}", [128, F], f32))
              for i in range(NR)]
        pb = [ctx.enter_context(nc.sbuf_tensor(f"p{i}", [128, F], f32))
              for i in range(NR)]
        qa = [ctx.enter_context(nc.sbuf_tensor(f"qa{i}", [128, F], f32))
              for i in range(NR)]
        pad = ctx.enter_context(nc.sbuf_tensor("pad", [128, F], f32))

        off = 0
        for i, n in enumerate(DMASLICES):
            w = (AUXW if i == 0 else 0) + 2 * n * F
            nc.sync.dma_start(blob[i][:, :], blob_in[:, off:off + w]) \
                .then_inc(sems[i], 16)
            off += w

        aux = blob[0]

        def hcol(j):
            s, o = divmod(j, SEG)
            return hseg[s][:, o * F:(o + 1) * F]

        def sl_of(j):
            for i in range(nsl - 1, -1, -1):
                if j >= sl_start[i]:
                    return i

        def y2col(j):
            s = sl_of(j)
            o = j - sl_start[s]
            base = AUXW if s == 0 else 0
            return blob[s][:, base + o * F:base + (o + 1) * F]

        def ddcol(j):
            s = sl_of(j)
            o = j - sl_start[s]
            base = (AUXW if s == 0 else 0) + DMASLICES[s] * F
            return blob[s][:, base + o * F:base + (o + 1) * F]

        # init: h_0 and Q_{-1} (pad memset keeps first-step RAW distances >= 2)
        nc.vector.wait_ge(sems[0], 16)
        nc.vector.tensor_copy(hcol(0), aux[:, 0:F])
        nc.vector.tensor_copy(qb[(NQ - 1) % NQ][:, :], aux[:, F:2 * F])
        nc.vector.memset(pad[:, :], 0.0)

        for j in range(NSTEP):
            if j in (sl_start[1], sl_start[2]):
                nc.vector.wait_ge(sems[sl_of(j)], 16)
            Hj = hcol(j)
            Hn = hcol(j + 1)
            Qp = qb[(j - 1) % NQ][:, :]
            Qn = qb[j % NQ][:, :]
            r = rb[j % NR][:, :]
            m = mb[j % NR][:, :]
            P = pb[j % NR][:, :]
            Qa = qa[j % NR][:, :]
            # ring [r, P, m, Qa, Hn, Qn]: every RAW dep >= 2 instructions back
            nc.vector.reciprocal_approx_fast(r, Hj)
            nc.vector.scalar_tensor_tensor(P, Hj, k1, Qp, mult, add)
            nc.vector.scalar_tensor_tensor(m, r, 1.0, y2col(j), mult, mult)
            nc.vector.scalar_tensor_tensor(Qa, Hj, gam, ddcol(j), mult, add)
            nc.vector.scalar_tensor_tensor(Hn, m, 1.0, P, mult, add)
            nc.vector.scalar_tensor_tensor(Qn, Qp, nu, Qa, mult, add)

        # layout fix: h columns W..W+C-1 all live in hseg[W//SEG]
        # (W % SEG + C <= SEG); gather per lane f into contiguous hout cols.
        s0, o0 = divmod(W, SEG)
        assert o0 + C <= SEG
        src_seg = hseg[s0][:, :].rearrange("p (t f) -> p t f", f=F)
        for f in range(F):
            inst = nc.vector.tensor_copy(hout[:, f * C:(f + 1) * C],
                                         src_seg[:, o0:o0 + C, f])
        inst.then_inc(csem, 1)
        nc.sync.wait_ge(csem, 1)
        nc.sync.dma_start(out[:, :], hout[:, :]).then_inc(sems[0], 16)
    nc.finalize()
    return nc


def _prep_inputs(y, omega, alpha, phi, lam, gam1, gam2, vphi, rho):
    """Host-side per-core input construction (fp64 intermediate)."""
    y = np.asarray(y, dtype=np.float32)
    bA = (1 - phi) * vphi + alpha
    bu = -2 * ((1 - phi) * vphi * gam2 + alpha * gam1)
    c1 = phi + rho + bA * lam**2 - bu * lam
    c2 = -rho * (phi + alpha * lam**2 + 2 * alpha * gam1 * lam)
    c4 = -rho * alpha
    K2 = (1 - phi) * (1 - rho) * omega - (1 - phi) * vphi - alpha * (1 - rho)
    e1 = bu - 2 * bA * lam
    e2 = 2 * rho * alpha * (lam + gam1)
    nu = -c4 / bA
    k1 = c1 - nu
    gam = c2 + nu * k1
    Kc = (1 - phi) * omega * (1 - rho) - (1 - phi) * vphi - alpha
    cP = phi + bA * lam**2 - bu * lam

    q0 = float(np.var(y.astype(np.float64)))
    yq = y.astype(np.float64)
    y2 = yq * yq

    # global lane table: lane g = (core*128 + p)*F + f ; chunkstart = g*C
    G = NCORES * 128 * F
    s = np.arange(G) * C
    j = np.arange(NSTEP)
    iy = s[:, None] - W + j[None, :]          # [G, NSTEP]
    iy_c = np.clip(iy, 0, T - 1)
    iy1_c = np.clip(iy + 1, 0, T - 1)
    Y2 = (bA * y2[iy_c]).astype(np.float32)
    DD = (e1 * yq[iy1_c] + e2 * yq[iy_c] + K2).astype(np.float32)

    Pstar = q0 * (1 - bA)
    Qstar = Pstar - k1 * q0
    Dstar = Qstar * (1 - nu) - gam * q0
    syn = iy < -1
    Y2[syn] = np.float32(bA * q0 * q0)
    DD[syn] = np.float32(Dstar)
    tr = iy == -1
    Y2[tr] = np.float32(bA * q0 * q0)
    P0_exact = cP * q0 + (1 - phi) * rho * q0 + e1 * yq[0] + Kc
    D0_craft = (P0_exact - k1 * q0) - gam * q0 - nu * Qstar
    DD[tr] = np.float32(D0_craft)

    iy0 = s - W
    Pinit = np.where(iy0 >= 0,
                     cP * q0 + (1 - phi) * rho * q0 + e1 * yq[np.clip(iy0, 0, T - 1)] + Kc,
                     Pstar)
    Qinit = (Pinit - k1 * q0).astype(np.float32)
    hinit = np.full(G, q0, dtype=np.float32)

    # reshape to per-core, per-partition, j-major-free layout
    Y2 = Y2.reshape(NCORES, 128, F, NSTEP).transpose(0, 1, 3, 2).reshape(
        NCORES, 128, NSTEP * F)
    DD = DD.reshape(NCORES, 128, F, NSTEP).transpose(0, 1, 3, 2).reshape(
        NCORES, 128, NSTEP * F)
    hinit = hinit.reshape(NCORES, 128, F)
    Qinit = Qinit.reshape(NCORES, 128, F)

    in_maps = []
    for k in range(NCORES):
        aux = np.empty((128, 2 * F + 3), dtype=np.float32)
        aux[:, 0:F] = hinit[k]
        aux[:, F:2 * F] = Qinit[k]
        aux[:, 2 * F] = np.float32(k1)
        aux[:, 2 * F + 1] = np.float32(nu)
        aux[:, 2 * F + 2] = np.float32(gam)
        AUXW = 2 * F + 3
        blobk = np.empty((128, AUXW + 2 * NSTEP * F), dtype=np.float32)
        blobk[:, :AUXW] = aux
        off = AUXW
        jlo = 0
        for n in DMASLICES:
            blobk[:, off:off + n * F] = Y2[k][:, jlo * F:(jlo + n) * F]
            off += n * F
            blobk[:, off:off + n * F] = DD[k][:, jlo * F:(jlo + n) * F]
            off += n * F
            jlo += n
        in_maps.append({"blob": blobk})
    return in_maps, np.float32(q0)


def kernel(y, omega, alpha, phi, lam, gam1, gam2, vphi, rho, _timing=None):
    from concourse.bass_utils import run_bass_kernel_spmd

    in_maps, q0 = _prep_inputs(
        y, float(omega), float(alpha), float(phi), float(lam),
        float(gam1), float(gam2), float(vphi), float(rho))

    if "nc" not in _cache:
        bA = (1 - float(phi)) * float(vphi) + float(alpha)
        bu = -2 * ((1 - float(phi)) * float(vphi) * float(gam2)
                   + float(alpha) * float(gam1))
        c1 = float(phi) + float(rho) + bA * float(lam)**2 - bu * float(lam)
        c2 = -float(rho) * (float(phi) + float(alpha) * float(lam)**2
                            + 2 * float(alpha) * float(gam1) * float(lam))
        c4 = -float(rho) * float(alpha)
        nuv = -c4 / bA
        k1v = c1 - nuv
        gamv = c2 + nuv * k1v
        _cache["nc"] = _build(float(np.float32(k1v)), float(np.float32(nuv)),
                              float(np.float32(gamv)))
    nc = _cache["nc"]

    trace = _timing is not None
    res = run_bass_kernel_spmd(nc, in_maps, core_ids=list(range(NCORES)),
                               trace=trace)
    if trace:
        _timing["exec_time_ns"] = res.exec_time_ns

    outp = np.empty(T, dtype=np.float32)
    for k in range(NCORES):
        outp[k * (T // NCORES):(k + 1) * (T // NCORES)] = \
            res.results[k]["o"].reshape(-1)
    outp[0] = q0
    return outp


# revision 13
# speedup vs baseline: 2.4217x; 1.0845x over previous
"""Component Heston-Nandi GARCH volatility recurrence on 8 Trainium2 cores.

Strategy: the (h,q) recurrence is strongly contracting (~0.983/step), so the
1M-step sequential scan is split into 16384 chunks of C=64 steps, each
computed by one SIMD lane (8 cores x 128 partitions x F=16 free lanes).  Each
lane warms up for W=320 steps from a stationary initial guess before its
chunk starts (host-validated max rel err 7.6e-3 vs the 2e-2 gate).  Lanes
whose chunk starts before position W start *exactly* at t=0 via synthetic
fixed-point warmup data.

The q-state is eliminated algebraically (see _prep_inputs) giving per step:
    h_{t+1} = bA*y_t^2 * (1/h_t) + P_t ;  P_t = k1*h_t + Q_{t-1}
    Q_t     = gam*h_t + nu*Q_{t-1} + D_t

Scheduling: hand-authored instruction stream on the Vector engine with NO
per-op semaphores.  The DVE pipeline does not interlock same-engine RAW
hazards, but a probe (proto/probe.py) shows one intervening instruction
(distance >= 2) makes reads bit-exact.  The 6-op ring
    [r, P, m, Qa, Hn, Qn]
has every RAW dependency at distance >= 2, so the only semaphores are the
DMA-slice handshakes.  This runs ~2x faster than the semaphore-synced
schedule (per-op waits were 67-230 ns each).
"""
import numpy as np

T = 1048576
NCORES = 8
F = 32           # lanes per partition (free dim)
C = T // (NCORES * 128 * F)   # chunk length per lane (=32)
W = 288          # warmup steps (host-validated: max rel 1.22e-2 < 2e-2 gate)
NSTEP = W + C - 1
SEG = 64         # steps per h ring segment (W % SEG + C <= SEG)
DMASLICES = [8, 128, NSTEP - 136]   # input slices: small first for fast start

_cache = {}


def _build(k1, nu, gam):
    import concourse.bacc as bacc
    import concourse.mybir as mybir
    from contextlib import ExitStack

    f32 = mybir.dt.float32
    add = mybir.AluOpType.add
    mult = mybir.AluOpType.mult

    nc = bacc.Bacc("TRN2", target_bir_lowering=False, debug=False,
                   num_devices=NCORES)
    AUXW = 2 * F + 3
    blob_in = nc.dram_tensor("blob", [128, AUXW + 2 * NSTEP * F], f32,
                             kind="ExternalInput")
    out = nc.dram_tensor("o", [128, F * C], f32, kind="ExternalOutput")

    nseg = (NSTEP + SEG) // SEG   # h columns 0..NSTEP inclusive
    nsl = len(DMASLICES)
    sl_start = [0] * nsl
    for i in range(1, nsl):
        sl_start[i] = sl_start[i - 1] + DMASLICES[i - 1]

    NQ = 8
    NR = 4
    with ExitStack() as ctx:
        sems = [ctx.enter_context(nc.semaphore(f"ds{i}")) for i in range(nsl)]
        csem = ctx.enter_context(nc.semaphore("csem"))
        blob = [ctx.enter_context(nc.sbuf_tensor(
            f"blob{i}", [128, (AUXW if i == 0 else 0) + 2 * n * F], f32))
            for i, n in enumerate(DMASLICES)]
        hseg = [ctx.enter_context(nc.sbuf_tensor(f"h{i}", [128, SEG * F], f32))
                for i in range(nseg)]
        qb = [ctx.enter_context(nc.sbuf_tensor(f"q{i}", [128, F], f32))
              for i in range(NQ)]
        rb = [ctx.enter_context(nc.sbuf_tensor(f"r{i}", [128, F], f32))
              for i in range(NR)]
        mb = [ctx.enter_context(nc.sbuf_tensor(f"m{i}", [128, F], f32))
              for i in range(NR)]
        pb = [ctx.enter_context(nc.sbuf_tensor(f"p{i}", [128, F], f32))
              for i in range(NR)]
        qa = [ctx.enter_context(nc.sbuf_tensor(f"qa{i}", [128, F], f32))
              for i in range(NR)]
        pad = ctx.enter_context(nc.sbuf_tensor("pad", [128, F], f32))

        off = 0
        for i, n in enumerate(DMASLICES):
            w = (AUXW if i == 0 else 0) + 2 * n * F
            nc.sync.dma_start(blob[i][:, :], blob_in[:, off:off + w]) \
                .then_inc(sems[i], 16)
            off += w

        aux = blob[0]

        def hcol(j):
            s, o = divmod(j, SEG)
            return hseg[s][:, o * F:(o + 1) * F]

        def sl_of(j):
            for i in range(nsl - 1, -1, -1):
                if j >= sl_start[i]:
                    return i

        def y2col(j):
            s = sl_of(j)
            o = j - sl_start[s]
            base = AUXW if s == 0 else 0
            return blob[s][:, base + o * F:base + (o + 1) * F]

        def ddcol(j):
            s = sl_of(j)
            o = j - sl_start[s]
            base = (AUXW if s == 0 else 0) + DMASLICES[s] * F
            return blob[s][:, base + o * F:base + (o + 1) * F]

        # init: h_0 and Q_{-1} (pad memset keeps first-step RAW distances >= 2)
        nc.vector.wait_ge(sems[0], 16)
        nc.vector.tensor_copy(hcol(0), aux[:, 0:F])
        nc.vector.tensor_copy(qb[(NQ - 1) % NQ][:, :], aux[:, F:2 * F])
        nc.vector.memset(pad[:, :], 0.0)

        for j in range(NSTEP):
            if j in (sl_start[1], sl_start[2]):
                nc.vector.wait_ge(sems[sl_of(j)], 16)
            Hj = hcol(j)
            Hn = hcol(j + 1)
            Qp = qb[(j - 1) % NQ][:, :]
            Qn = qb[j % NQ][:, :]
            r = rb[j % NR][:, :]
            m = mb[j % NR][:, :]
            P = pb[j % NR][:, :]
            Qa = qa[j % NR][:, :]
            # ring [r, P, m, Qa, Hn, Qn]: every RAW dep >= 2 instructions back
            nc.vector.reciprocal_approx_fast(r, Hj)
            nc.vector.scalar_tensor_tensor(P, Hj, k1, Qp, mult, add)
            nc.vector.scalar_tensor_tensor(m, r, 1.0, y2col(j), mult, mult)
            nc.vector.scalar_tensor_tensor(Qa, Hj, gam, ddcol(j), mult, add)
            nc.vector.scalar_tensor_tensor(Hn, m, 1.0, P, mult, add)
            inst = nc.vector.scalar_tensor_tensor(Qn, Qp, nu, Qa, mult, add)
        inst.then_inc(csem, 1)

        # h columns W..W+C-1 live contiguously in hseg[W//SEG] as [t, f];
        # DMA them out directly — the host undoes the (C, F) interleave.
        s0, o0 = divmod(W, SEG)
        assert o0 + C <= SEG
        nc.sync.wait_ge(csem, 1)
        nc.sync.dma_start(out[:, :], hseg[s0][:, o0 * F:(o0 + C) * F]) \
            .then_inc(sems[0], 16)
    nc.finalize()
    return nc


def _prep_inputs(y, omega, alpha, phi, lam, gam1, gam2, vphi, rho):
    """Host-side per-core input construction (fp64 intermediate)."""
    y = np.asarray(y, dtype=np.float32)
    bA = (1 - phi) * vphi + alpha
    bu = -2 * ((1 - phi) * vphi * gam2 + alpha * gam1)
    c1 = phi + rho + bA * lam**2 - bu * lam
    c2 = -rho * (phi + alpha * lam**2 + 2 * alpha * gam1 * lam)
    c4 = -rho * alpha
    K2 = (1 - phi) * (1 - rho) * omega - (1 - phi) * vphi - alpha * (1 - rho)
    e1 = bu - 2 * bA * lam
    e2 = 2 * rho * alpha * (lam + gam1)
    nu = -c4 / bA
    k1 = c1 - nu
    gam = c2 + nu * k1
    Kc = (1 - phi) * omega * (1 - rho) - (1 - phi) * vphi - alpha
    cP = phi + bA * lam**2 - bu * lam

    q0 = float(np.var(y.astype(np.float64)))
    yq = y.astype(np.float64)
    y2 = yq * yq

    # global lane table: lane g = (core*128 + p)*F + f ; chunkstart = g*C
    G = NCORES * 128 * F
    s = np.arange(G) * C
    j = np.arange(NSTEP)
    iy = s[:, None] - W + j[None, :]          # [G, NSTEP]
    iy_c = np.clip(iy, 0, T - 1)
    iy1_c = np.clip(iy + 1, 0, T - 1)
    Y2 = (bA * y2[iy_c]).astype(np.float32)
    DD = (e1 * yq[iy1_c] + e2 * yq[iy_c] + K2).astype(np.float32)

    Pstar = q0 * (1 - bA)
    Qstar = Pstar - k1 * q0
    Dstar = Qstar * (1 - nu) - gam * q0
    syn = iy < -1
    Y2[syn] = np.float32(bA * q0 * q0)
    DD[syn] = np.float32(Dstar)
    tr = iy == -1
    Y2[tr] = np.float32(bA * q0 * q0)
    P0_exact = cP * q0 + (1 - phi) * rho * q0 + e1 * yq[0] + Kc
    D0_craft = (P0_exact - k1 * q0) - gam * q0 - nu * Qstar
    DD[tr] = np.float32(D0_craft)

    iy0 = s - W
    Pinit = np.where(iy0 >= 0,
                     cP * q0 + (1 - phi) * rho * q0 + e1 * yq[np.clip(iy0, 0, T - 1)] + Kc,
                     Pstar)
    Qinit = (Pinit - k1 * q0).astype(np.float32)
    hinit = np.full(G, q0, dtype=np.float32)

    # reshape to per-core, per-partition, j-major-free layout
    Y2 = Y2.reshape(NCORES, 128, F, NSTEP).transpose(0, 1, 3, 2).reshape(
        NCORES, 128, NSTEP * F)
    DD = DD.reshape(NCORES, 128, F, NSTEP).transpose(0, 1, 3, 2).reshape(
        NCORES, 128, NSTEP * F)
    hinit = hinit.reshape(NCORES, 128, F)
    Qinit = Qinit.reshape(NCORES, 128, F)

    in_maps = []
    for k in range(NCORES):
        aux = np.empty((128, 2 * F + 3), dtype=np.float32)
        aux[:, 0:F] = hinit[k]
        aux[:, F:2 * F] = Qinit[k]
        aux[:, 2 * F] = np.float32(k1)
        aux[:, 2 * F + 1] = np.float32(nu)
        aux[:, 2 * F + 2] = np.float32(gam)
        AUXW = 2 * F + 3
        blobk = np.empty((128, AUXW + 2 * NSTEP * F), dtype=np.float32)
        blobk[:, :AUXW] = aux
        off = AUXW
        jlo = 0
        for n in DMASLICES:
            blobk[:, off:off + n * F] = Y2[k][:, jlo * F:(jlo + n) * F]
            off += n * F
            blobk[:, off:off + n * F] = DD[k][:, jlo * F:(jlo + n) * F]
            off += n * F
            jlo += n
        in_maps.append({"blob": blobk})
    return in_maps, np.float32(q0)


def kernel(y, omega, alpha, phi, lam, gam1, gam2, vphi, rho, _timing=None):
    from concourse.bass_utils import run_bass_kernel_spmd

    in_maps, q0 = _prep_inputs(
        y, float(omega), float(alpha), float(phi), float(lam),
        float(gam1), float(gam2), float(vphi), float(rho))

    if "nc" not in _cache:
        bA = (1 - float(phi)) * float(vphi) + float(alpha)
        bu = -2 * ((1 - float(phi)) * float(vphi) * float(gam2)
                   + float(alpha) * float(gam1))
        c1 = float(phi) + float(rho) + bA * float(lam)**2 - bu * float(lam)
        c2 = -float(rho) * (float(phi) + float(alpha) * float(lam)**2
                            + 2 * float(alpha) * float(gam1) * float(lam))
        c4 = -float(rho) * float(alpha)
        nuv = -c4 / bA
        k1v = c1 - nuv
        gamv = c2 + nuv * k1v
        _cache["nc"] = _build(float(np.float32(k1v)), float(np.float32(nuv)),
                              float(np.float32(gamv)))
    nc = _cache["nc"]

    trace = _timing is not None
    res = run_bass_kernel_spmd(nc, in_maps, core_ids=list(range(NCORES)),
                               trace=trace)
    if trace:
        _timing["exec_time_ns"] = res.exec_time_ns

    outp = np.empty(T, dtype=np.float32)
    for k in range(NCORES):
        # device layout is [p, t, f]; lane-major order is [p, f, t]
        outp[k * (T // NCORES):(k + 1) * (T // NCORES)] = \
            res.results[k]["o"].reshape(128, C, F).transpose(0, 2, 1).reshape(-1)
    outp[0] = q0
    return outp


# revision 15
# speedup vs baseline: 2.5014x; 1.0329x over previous
"""Component Heston-Nandi GARCH volatility recurrence on 8 Trainium2 cores.

Strategy: the (h,q) recurrence is strongly contracting (~0.983/step), so the
1M-step sequential scan is split into 16384 chunks of C=64 steps, each
computed by one SIMD lane (8 cores x 128 partitions x F=16 free lanes).  Each
lane warms up for W=320 steps from a stationary initial guess before its
chunk starts (host-validated max rel err 7.6e-3 vs the 2e-2 gate).  Lanes
whose chunk starts before position W start *exactly* at t=0 via synthetic
fixed-point warmup data.

The q-state is eliminated algebraically (see _prep_inputs) giving per step:
    h_{t+1} = bA*y_t^2 * (1/h_t) + P_t ;  P_t = k1*h_t + Q_{t-1}
    Q_t     = gam*h_t + nu*Q_{t-1} + D_t

Scheduling: hand-authored instruction stream on the Vector engine with NO
per-op semaphores.  The DVE pipeline does not interlock same-engine RAW
hazards, but a probe (proto/probe.py) shows one intervening instruction
(distance >= 2) makes reads bit-exact.  The 6-op ring
    [r, P, m, Qa, Hn, Qn]
has every RAW dependency at distance >= 2, so the only semaphores are the
DMA-slice handshakes.  This runs ~2x faster than the semaphore-synced
schedule (per-op waits were 67-230 ns each).
"""
import numpy as np

T = 1048576
NCORES = 8
F = 32           # lanes per partition (free dim)
C = T // (NCORES * 128 * F)   # chunk length per lane (=32)
W = 288          # warmup steps (host-validated: max rel 1.22e-2 < 2e-2 gate)
NSTEP = W + C - 1
SEG = 64         # steps per h ring segment (W % SEG + C <= SEG)
DMASLICES = [8, 48, NSTEP - 56]   # input slices: sized so each lands in time

_cache = {}


def _build(k1, nu, gam):
    import concourse.bacc as bacc
    import concourse.mybir as mybir
    from contextlib import ExitStack

    f32 = mybir.dt.float32
    add = mybir.AluOpType.add
    mult = mybir.AluOpType.mult

    nc = bacc.Bacc("TRN2", target_bir_lowering=False, debug=False,
                   num_devices=NCORES)
    AUXW = 2 * F + 3
    blob_in = nc.dram_tensor("blob", [128, AUXW + 2 * NSTEP * F], f32,
                             kind="ExternalInput")
    out = nc.dram_tensor("o", [128, F * C], f32, kind="ExternalOutput")

    nseg = (NSTEP + SEG) // SEG   # h columns 0..NSTEP inclusive
    nsl = len(DMASLICES)
    sl_start = [0] * nsl
    for i in range(1, nsl):
        sl_start[i] = sl_start[i - 1] + DMASLICES[i - 1]

    NQ = 8
    NR = 4
    with ExitStack() as ctx:
        sems = [ctx.enter_context(nc.semaphore(f"ds{i}")) for i in range(nsl)]
        csem = ctx.enter_context(nc.semaphore("csem"))
        blob = [ctx.enter_context(nc.sbuf_tensor(
            f"blob{i}", [128, (AUXW if i == 0 else 0) + 2 * n * F], f32))
            for i, n in enumerate(DMASLICES)]
        hseg = [ctx.enter_context(nc.sbuf_tensor(f"h{i}", [128, SEG * F], f32))
                for i in range(nseg)]
        qb = [ctx.enter_context(nc.sbuf_tensor(f"q{i}", [128, F], f32))
              for i in range(NQ)]
        rb = [ctx.enter_context(nc.sbuf_tensor(f"r{i}", [128, F], f32))
              for i in range(NR)]
        mb = [ctx.enter_context(nc.sbuf_tensor(f"m{i}", [128, F], f32))
              for i in range(NR)]
        pb = [ctx.enter_context(nc.sbuf_tensor(f"p{i}", [128, F], f32))
              for i in range(NR)]
        qa = [ctx.enter_context(nc.sbuf_tensor(f"qa{i}", [128, F], f32))
              for i in range(NR)]
        pad = ctx.enter_context(nc.sbuf_tensor("pad", [128, F], f32))

        off = 0
        for i, n in enumerate(DMASLICES):
            w = (AUXW if i == 0 else 0) + 2 * n * F
            nc.sync.dma_start(blob[i][:, :], blob_in[:, off:off + w]) \
                .then_inc(sems[i], 16)
            off += w

        aux = blob[0]

        def hcol(j):
            s, o = divmod(j, SEG)
            return hseg[s][:, o * F:(o + 1) * F]

        def sl_of(j):
            for i in range(nsl - 1, -1, -1):
                if j >= sl_start[i]:
                    return i

        def y2col(j):
            s = sl_of(j)
            o = j - sl_start[s]
            base = AUXW if s == 0 else 0
            return blob[s][:, base + o * F:base + (o + 1) * F]

        def ddcol(j):
            s = sl_of(j)
            o = j - sl_start[s]
            base = (AUXW if s == 0 else 0) + DMASLICES[s] * F
            return blob[s][:, base + o * F:base + (o + 1) * F]

        # init: h_0 and Q_{-1} (pad memset keeps first-step RAW distances >= 2)
        nc.vector.wait_ge(sems[0], 16)
        nc.vector.tensor_copy(hcol(0), aux[:, 0:F])
        nc.vector.tensor_copy(qb[(NQ - 1) % NQ][:, :], aux[:, F:2 * F])
        nc.vector.memset(pad[:, :], 0.0)

        for j in range(NSTEP):
            if j in (sl_start[1], sl_start[2]):
                nc.vector.wait_ge(sems[sl_of(j)], 16)
            Hj = hcol(j)
            Hn = hcol(j + 1)
            Qp = qb[(j - 1) % NQ][:, :]
            Qn = qb[j % NQ][:, :]
            r = rb[j % NR][:, :]
            m = mb[j % NR][:, :]
            P = pb[j % NR][:, :]
            Qa = qa[j % NR][:, :]
            # ring [r, P, m, Qa, Hn, Qn]: every RAW dep >= 2 instructions back
            nc.vector.reciprocal_approx_fast(r, Hj)
            nc.vector.scalar_tensor_tensor(P, Hj, k1, Qp, mult, add)
            nc.vector.tensor_mul(m, r, y2col(j))
            nc.vector.scalar_tensor_tensor(Qa, Hj, gam, ddcol(j), mult, add)
            nc.vector.tensor_add(Hn, m, P)
            inst = nc.vector.scalar_tensor_tensor(Qn, Qp, nu, Qa, mult, add)
        inst.then_inc(csem, 1)

        # h columns W..W+C-1 live contiguously in hseg[W//SEG] as [t, f];
        # DMA them out directly — the host undoes the (C, F) interleave.
        s0, o0 = divmod(W, SEG)
        assert o0 + C <= SEG
        nc.sync.wait_ge(csem, 1)
        nc.sync.dma_start(out[:, :], hseg[s0][:, o0 * F:(o0 + C) * F]) \
            .then_inc(sems[0], 16)
    nc.finalize()
    return nc


def _prep_inputs(y, omega, alpha, phi, lam, gam1, gam2, vphi, rho):
    """Host-side per-core input construction (fp64 intermediate)."""
    y = np.asarray(y, dtype=np.float32)
    bA = (1 - phi) * vphi + alpha
    bu = -2 * ((1 - phi) * vphi * gam2 + alpha * gam1)
    c1 = phi + rho + bA * lam**2 - bu * lam
    c2 = -rho * (phi + alpha * lam**2 + 2 * alpha * gam1 * lam)
    c4 = -rho * alpha
    K2 = (1 - phi) * (1 - rho) * omega - (1 - phi) * vphi - alpha * (1 - rho)
    e1 = bu - 2 * bA * lam
    e2 = 2 * rho * alpha * (lam + gam1)
    nu = -c4 / bA
    k1 = c1 - nu
    gam = c2 + nu * k1
    Kc = (1 - phi) * omega * (1 - rho) - (1 - phi) * vphi - alpha
    cP = phi + bA * lam**2 - bu * lam

    q0 = float(np.var(y.astype(np.float64)))
    yq = y.astype(np.float64)
    y2 = yq * yq

    # global lane table: lane g = (core*128 + p)*F + f ; chunkstart = g*C
    G = NCORES * 128 * F
    s = np.arange(G) * C
    j = np.arange(NSTEP)
    iy = s[:, None] - W + j[None, :]          # [G, NSTEP]
    iy_c = np.clip(iy, 0, T - 1)
    iy1_c = np.clip(iy + 1, 0, T - 1)
    Y2 = (bA * y2[iy_c]).astype(np.float32)
    DD = (e1 * yq[iy1_c] + e2 * yq[iy_c] + K2).astype(np.float32)

    Pstar = q0 * (1 - bA)
    Qstar = Pstar - k1 * q0
    Dstar = Qstar * (1 - nu) - gam * q0
    syn = iy < -1
    Y2[syn] = np.float32(bA * q0 * q0)
    DD[syn] = np.float32(Dstar)
    tr = iy == -1
    Y2[tr] = np.float32(bA * q0 * q0)
    P0_exact = cP * q0 + (1 - phi) * rho * q0 + e1 * yq[0] + Kc
    D0_craft = (P0_exact - k1 * q0) - gam * q0 - nu * Qstar
    DD[tr] = np.float32(D0_craft)

    iy0 = s - W
    Pinit = np.where(iy0 >= 0,
                     cP * q0 + (1 - phi) * rho * q0 + e1 * yq[np.clip(iy0, 0, T - 1)] + Kc,
                     Pstar)
    Qinit = (Pinit - k1 * q0).astype(np.float32)
    hinit = np.full(G, q0, dtype=np.float32)

    # reshape to per-core, per-partition, j-major-free layout
    Y2 = Y2.reshape(NCORES, 128, F, NSTEP).transpose(0, 1, 3, 2).reshape(
        NCORES, 128, NSTEP * F)
    DD = DD.reshape(NCORES, 128, F, NSTEP).transpose(0, 1, 3, 2).reshape(
        NCORES, 128, NSTEP * F)
    hinit = hinit.reshape(NCORES, 128, F)
    Qinit = Qinit.reshape(NCORES, 128, F)

    in_maps = []
    for k in range(NCORES):
        aux = np.empty((128, 2 * F + 3), dtype=np.float32)
        aux[:, 0:F] = hinit[k]
        aux[:, F:2 * F] = Qinit[k]
        aux[:, 2 * F] = np.float32(k1)
        aux[:, 2 * F + 1] = np.float32(nu)
        aux[:, 2 * F + 2] = np.float32(gam)
        AUXW = 2 * F + 3
        blobk = np.empty((128, AUXW + 2 * NSTEP * F), dtype=np.float32)
        blobk[:, :AUXW] = aux
        off = AUXW
        jlo = 0
        for n in DMASLICES:
            blobk[:, off:off + n * F] = Y2[k][:, jlo * F:(jlo + n) * F]
            off += n * F
            blobk[:, off:off + n * F] = DD[k][:, jlo * F:(jlo + n) * F]
            off += n * F
            jlo += n
        in_maps.append({"blob": blobk})
    return in_maps, np.float32(q0)


def kernel(y, omega, alpha, phi, lam, gam1, gam2, vphi, rho, _timing=None):
    from concourse.bass_utils import run_bass_kernel_spmd

    in_maps, q0 = _prep_inputs(
        y, float(omega), float(alpha), float(phi), float(lam),
        float(gam1), float(gam2), float(vphi), float(rho))

    if "nc" not in _cache:
        bA = (1 - float(phi)) * float(vphi) + float(alpha)
        bu = -2 * ((1 - float(phi)) * float(vphi) * float(gam2)
                   + float(alpha) * float(gam1))
        c1 = float(phi) + float(rho) + bA * float(lam)**2 - bu * float(lam)
        c2 = -float(rho) * (float(phi) + float(alpha) * float(lam)**2
                            + 2 * float(alpha) * float(gam1) * float(lam))
        c4 = -float(rho) * float(alpha)
        nuv = -c4 / bA
        k1v = c1 - nuv
        gamv = c2 + nuv * k1v
        _cache["nc"] = _build(float(np.float32(k1v)), float(np.float32(nuv)),
                              float(np.float32(gamv)))
    nc = _cache["nc"]

    trace = _timing is not None
    res = run_bass_kernel_spmd(nc, in_maps, core_ids=list(range(NCORES)),
                               trace=trace)
    if trace:
        _timing["exec_time_ns"] = res.exec_time_ns

    outp = np.empty(T, dtype=np.float32)
    for k in range(NCORES):
        # device layout is [p, t, f]; lane-major order is [p, f, t]
        outp[k * (T // NCORES):(k + 1) * (T // NCORES)] = \
            res.results[k]["o"].reshape(128, C, F).transpose(0, 2, 1).reshape(-1)
    outp[0] = q0
    return outp
